# revision 61
# baseline (speedup 1.0000x reference)
import sys
import numpy as np
from contextlib import ExitStack

import os as _os_early
for _p in ("/opt/trn_rl_repo", "/root/.axon_site/_ro/trn_rl_repo"):
    if _os_early.path.isdir(_p) and _p not in sys.path:
        sys.path.insert(0, _p)
import concourse.bass as bass
import concourse.tile as tile
from concourse import mybir, bacc

F32 = mybir.dt.float32
F16 = mybir.dt.float16
EXP = mybir.ActivationFunctionType.Exp

D, L, NTR, H, DK, T = 256, 128, 64, 8, 32, 8192
CH_A = 8
CHW = 1024

W_NAMES = ["t_wq", "t_wk", "t_wv", "sq_w", "s_wk", "s_wv", "s_wd"]

_tile_ctr = [0]


def _mk(pool, shape, dtype, tag):
    _tile_ctr[0] += 1
    return pool.tile(shape, dtype, tag=tag, name=f"{tag}_{_tile_ctr[0]}")


def build_nc(phases="AB"):
    nc = bacc.Bacc("TRN2")
    qT = nc.declare_dram_parameter("qT", [D, T], F16, isOutput=False)
    kT = nc.declare_dram_parameter("kT", [D, T], F16, isOutput=False)
    vT = nc.declare_dram_parameter("vT", [D, T], F16, isOutput=False)
    vTtm = nc.declare_dram_parameter("vTtm", [D, T], F16, isOutput=False)
    w_dram = {n: nc.declare_dram_parameter(n, [D, D], F16, isOutput=False)
              for n in W_NAMES}
    sq_b = nc.declare_dram_parameter("sq_b", [D], F32, isOutput=False)
    s_bd = nc.declare_dram_parameter("s_bd", [D], F32, isOutput=False)
    outT = nc.declare_dram_parameter("outT", [D, T], F16, isOutput=True)

    with tile.TileContext(nc) as tc, ExitStack() as ctx:
        wpool = ctx.enter_context(tc.tile_pool(name="wpool", bufs=1))
        res_pool = ctx.enter_context(tc.tile_pool(name="res", bufs=1))

        w_sb = {}
        w_q = {"t_wq": nc.sync, "t_wk": nc.scalar, "s_wk": nc.scalar,
               "t_wv": nc.gpsimd, "sq_w": nc.gpsimd, "s_wv": nc.gpsimd,
               "s_wd": nc.gpsimd}
        for n in ("t_wq", "t_wk", "s_wk", "t_wv", "sq_w", "s_wv", "s_wd"):
            wt = _mk(wpool, [128, 2 * D], F16, f"w_{n}")
            w_q[n].dma_start(wt[:].rearrange("p (kt e) -> p kt e", kt=2),
                             w_dram[n].ap().rearrange("(kt p) e -> p kt e", p=128))
            w_sb[n] = [wt[:, 0:D], wt[:, D:2 * D]]
        sq_b_sb = _mk(wpool, [128, 2], F32, "sq_b")
        nc.gpsimd.dma_start(sq_b_sb[:], sq_b.ap().rearrange("(e p) -> p e", p=128))
        s_bd_sb = _mk(wpool, [128, 2], F32, "s_bd")
        nc.gpsimd.dma_start(s_bd_sb[:], s_bd.ap().rearrange("(e p) -> p e", p=128))
        ones_sb = _mk(wpool, [128, 32], F16, "ones")
        nc.vector.memset(ones_sb[:], 1.0)

        vres = [_mk(res_pool, [128, T], F16, f"vres{kt}") for kt in range(2)]

        def vres_slice(c):
            c0 = c * CHW
            for kt in range(2):
                nc.gpsimd.dma_start(vres[kt][:, c0:c0 + CHW],
                                    vT.ap()[kt * 128:(kt + 1) * 128, c0:c0 + CHW])

        s_kT_res = _mk(res_pool, [128, 2 * T], F16, "skT")
        s_qT_res = _mk(res_pool, [128, 2 * T], F16, "sqT")

        expP2 = []
        for i in range(5):
            ex = _mk(res_pool, [128, CHW], F16, f"exZ{i}")
            for ql in range(2):
                zv = ex[64 * (1 - ql):64 * (1 - ql) + 64, :].rearrange(
                    "p (h q) -> p h q", q=128)[:, :, ql * 64:ql * 64 + 64]
                nc.vector.memset(zv, 0.0)
            expP2.append(ex)

        def res_pm(t, g):
            return t[:, g * T:(g + 1) * T].rearrange("p (l n) -> p n l", n=NTR)

        if "A" not in phases:
            nc.vector.memset(s_kT_res[:], 0.0)
            nc.vector.memset(s_qT_res[:], 0.0)
        if "A" in phases:
         with ExitStack() as actx:
            a_in = actx.enter_context(tc.tile_pool(name="a_in", bufs=2))
            a_st = actx.enter_context(tc.tile_pool(name="a_st", bufs=2))
            a_tv = actx.enter_context(tc.tile_pool(name="a_tv", bufs=2))
            a_ta = actx.enter_context(tc.tile_pool(name="a_ta", bufs=2))
            a_exp = actx.enter_context(tc.tile_pool(name="a_exp", bufs=3))
            a_r = actx.enter_context(tc.tile_pool(name="a_r", bufs=3))
            a_h3 = actx.enter_context(tc.tile_pool(name="a_h3", bufs=2))
            sps = actx.enter_context(tc.tile_pool(name="sps", bufs=1, space="PSUM"))
            odp = actx.enter_context(tc.tile_pool(name="odp", bufs=2, space="PSUM"))
            ppj = actx.enter_context(tc.tile_pool(name="ppj", bufs=2, space="PSUM"))

            def dma_in(c):
                c0 = c * CHW
                out = []
                for nm, dram, eng in (("q", qT, nc.sync), ("k", kT, nc.sync),
                                      ("v", vTtm, nc.scalar)):
                    tile_ = _mk(a_in, [128, 2 * CHW], F16, nm)
                    eng.dma_start(
                        tile_[:].rearrange("p (kt c) -> p kt c", kt=2),
                        dram.ap()[:, c0:c0 + CHW].rearrange(
                            "(kt p) c -> p kt c", p=128))
                    out.append([tile_[:, 0:CHW], tile_[:, CHW:2 * CHW]])
                return out[0], out[1], out[2]

            def mk_staging():
                st_q = _mk(a_st, [128, 2048], F16, "stq")
                st_k = _mk(a_st, [128, 2048], F16, "stk")
                return st_q, st_k

            def _copy(eng, dst, src):
                if eng is nc.scalar:
                    eng.copy(dst, src)
                else:
                    eng.tensor_copy(dst, src)

            def proj_groups(c, q_sb, k_sb, st_q, st_k, holder, pools=None):
                groups = []
                ei = [0]
                evac_eng = [nc.scalar, nc.vector]
                if pools is None:
                    pools = (ppj,)
                ptags = {id(ppj): "ppj", id(odp): "od"}
                h3q = _mk(a_h3, [32, 2048], F16, "h3q")
                h3k = _mk(a_h3, [32, 2048], F16, "h3k")
                holder["h3"] = (h3q, h3k)

                def mk_tqtk(wname, xin, st, e, j):
                    def run():
                        pool = pools[ei[0] % len(pools)]
                        ps = _mk(pool, [128, 512], F32, ptags[id(pool)])
                        for kt in range(2):
                            nc.tensor.matmul(
                                ps[:],
                                w_sb[wname][kt][:, e * 128:(e + 1) * 128],
                                xin[kt][:, j * 512:(j + 1) * 512],
                                start=(kt == 0), stop=(kt == 1))
                        _copy(evac_eng[ei[0] % 2],
                              st[:, e * 1024 + j * 512: e * 1024 + (j + 1) * 512],
                              ps[:])
                        ei[0] += 1
                    return run

                def mk_h3(e):
                    def run():
                        nc.sync.dma_start(
                            h3q[0:32, e * 1024:(e + 1) * 1024],
                            st_q[96:128, e * 1024:(e + 1) * 1024])
                        nc.sync.dma_start(
                            h3k[0:32, e * 1024:(e + 1) * 1024],
                            st_k[96:128, e * 1024:(e + 1) * 1024])
                    return run

                def mk_swk(e, j):
                    def run():
                        pool = pools[(e * 2 + j) % len(pools)]
                        ps = _mk(pool, [128, 512], F32, ptags[id(pool)])
                        for kt in range(2):
                            nc.tensor.matmul(
                                ps[:],
                                w_sb["s_wk"][kt][:, e * 128:(e + 1) * 128],
                                k_sb[kt][:, j * 512:(j + 1) * 512],
                                start=(kt == 0), stop=(kt == 1))
                        nc.scalar.copy(
                            res_pm(s_kT_res, e)[:, 8 * c + 4 * j: 8 * c + 4 * j + 4, :],
                            ps[:].rearrange("p (n l) -> p n l", l=L))
                    return run

                for e in range(2):
                    for j in range(2):
                        groups.append(mk_tqtk("t_wq", q_sb, st_q, e, j))
                    for j in range(2):
                        groups.append(mk_tqtk("t_wk", k_sb, st_k, e, j))
                    groups.append(mk_h3(e))
                for e in range(2):
                    for j in range(2):
                        groups.append(mk_swk(e, j))
                return groups

            def sq_groups(c, t_att):
                groups = []

                def mk(e, j):
                    def run():
                        ps = _mk(ppj, [128, 512], F32, "ppj")
                        for kt in range(2):
                            nc.tensor.matmul(
                                ps[:],
                                w_sb["sq_w"][kt][:, e * 128:(e + 1) * 128],
                                t_att[:, kt * 1024 + j * 512: kt * 1024 + (j + 1) * 512],
                                start=(kt == 0), stop=(kt == 1))
                        nc.vector.tensor_scalar_add(
                            res_pm(s_qT_res, e)[:, 8 * c + 4 * j: 8 * c + 4 * j + 4, :],
                            ps[:].rearrange("p (n l) -> p n l", l=L),
                            sq_b_sb[:, e:e + 1])
                    return run

                for j in range(2):
                    for e in range(2):
                        groups.append(mk(e, j))
                return groups

            def chunk_attention(c, st_q, st_k, h3q, h3k, vtm_sb, pending,
                                late_factory=None):
                tv_all = _mk(a_tv, [128, 8 * 256], F16, "tv")
                t_att = _mk(a_ta, [128, 2048], F16, "ta")
                late = late_factory(t_att) if late_factory else {}
                expP = {}
                pss = _mk(sps, [128, 2048], F32, "S")

                def emit_front(t):
                    expP[t] = _mk(a_exp, [128, CHW], F16, "expP")
                    tvps = _mk(ppj, [128, 256], F32, "ppj")
                    par = t % 2

                    def score_quad(e):
                        for hh in range(4):
                            if hh < 3:
                                lhsT = st_k[32 * hh:32 * hh + 32,
                                            e * 1024 + t * 128: e * 1024 + (t + 1) * 128]
                                rhs = st_q[32 * hh:32 * hh + 32,
                                           e * 1024 + t * 128: e * 1024 + (t + 1) * 128]
                            else:
                                lhsT = h3k[0:32, e * 1024 + t * 128: e * 1024 + (t + 1) * 128]
                                rhs = h3q[0:32, e * 1024 + t * 128: e * 1024 + (t + 1) * 128]
                            base = hh * 512 + par * 256 + e * 128
                            nc.tensor.matmul(pss[:, base: base + 128], lhsT, rhs)

                    score_quad(0)
                    score_quad(1)
                    srcv = pss[:].rearrange("p (hh x) -> p hh x", hh=4)[
                        :, :, par * 256:par * 256 + 256].rearrange(
                        "p hh (e q) -> p hh e q", q=128)
                    dstv = expP[t][:].rearrange(
                        "p (e hh q) -> p hh e q", e=2, q=128)
                    nc.scalar.activation(dstv, srcv, EXP)
                    for kt in range(2):
                        nc.tensor.matmul(tvps[:],
                                         vtm_sb[kt][:, t * 128:(t + 1) * 128],
                                         w_sb["t_wv"][kt][:, :],
                                         start=(kt == 0), stop=(kt == 1))
                    if t % 2 == 0:
                        nc.scalar.copy(tv_all[:, t * 256:(t + 1) * 256], tvps[:])
                    else:
                        nc.vector.tensor_copy(tv_all[:, t * 256:(t + 1) * 256], tvps[:])

                def emit_back(u):
                    psod = _mk(odp, [128, 512], F32, "od")
                    exp_u = expP.pop(u)
                    for cc in range(4):
                        rhs = exp_u[:].rearrange("p (e x) -> p e x", e=2)[
                            :, :, cc * 128:(cc + 1) * 128]
                        nc.tensor.matmul(psod[32 * cc:32 * cc + 32, 256:512],
                                         ones_sb[:, 0:32], rhs,
                                         tile_position=(0, 32 * cc))
                    for e in range(2):
                        for hh in range(4):
                            h = 4 * e + hh
                            nc.tensor.matmul(
                                psod[32 * hh:32 * hh + 32, e * 128:(e + 1) * 128],
                                tv_all[:, u * 256 + h * 32: u * 256 + (h + 1) * 32],
                                exp_u[:, e * 512 + hh * 128: e * 512 + (hh + 1) * 128],
                                tile_position=(0, 32 * hh))
                    rec = _mk(a_r, [128, 256], F32, "rec")
                    nc.vector.reciprocal(rec[:], psod[:, 256:512])
                    dst = t_att[:].rearrange("p (g x) -> p g x", g=2)[
                        :, :, u * 128:(u + 1) * 128]
                    po = psod[:, 0:256].rearrange("p (g q) -> p g q", g=2)
                    rc = rec[:].rearrange("p (g q) -> p g q", g=2)
                    nc.vector.tensor_mul(dst, po, rc)

                for t in range(10):
                    if t < 8:
                        emit_front(t)
                    for _ in range(2):
                        if pending:
                            pending.pop(0)()
                    if t >= 2:
                        emit_back(t - 2)
                    for g in late.pop(t, []):
                        g()
                while pending:
                    pending.pop(0)()
                return t_att

            q_sb, k_sb, vtm_sb = dma_in(0)
            st_q, st_k = mk_staging()
            hold = {}
            for g in proj_groups(0, q_sb, k_sb, st_q, st_k, hold,
                                 pools=(ppj, odp)):
                g()
            h3q, h3k = hold["h3"]
            prev_ta = None
            for c in range(CH_A):
                if c + 1 < CH_A:
                    q2, k2, v2 = dma_in(c + 1)
                    st_q2, st_k2 = mk_staging()
                    hold2 = {}
                    pending = []
                    if prev_ta is not None:
                        pending += sq_groups(c - 1, prev_ta)
                    pending += proj_groups(c + 1, q2, k2, st_q2, st_k2, hold2)
                else:
                    pending = sq_groups(c - 1, prev_ta) if prev_ta is not None else []
                pending.append(lambda c=c: vres_slice(c))
                lf = None
                if c == CH_A - 1:
                    def lf(ta):
                        g = sq_groups(c, ta)
                        return {6: [g[0]], 7: [g[1]], 9: [g[2], g[3]]}
                ta = chunk_attention(c, st_q, st_k, h3q, h3k, vtm_sb, pending,
                                     late_factory=lf)
                if c + 1 < CH_A:
                    h3q, h3k = hold2["h3"]
                    st_q, st_k = st_q2, st_k2
                    q_sb, k_sb, vtm_sb = q2, k2, v2
                prev_ta = ta

        if "B" not in phases:
            zz = _mk(res_pool, [128, 1024], F16, "zz")
            nc.vector.memset(zz[:], 0.0)
            for e in range(2):
                for jj in range(8):
                    nc.sync.dma_start(
                        outT.ap()[e * 128:(e + 1) * 128, jj * 1024:(jj + 1) * 1024], zz[:])
        if "B" in phases:
         with ExitStack() as bctx:
            b_sv = bctx.enter_context(tc.tile_pool(name="b_sv", bufs=3))
            b_att = bctx.enter_context(tc.tile_pool(name="b_att", bufs=2))
            b_out = bctx.enter_context(tc.tile_pool(name="b_out", bufs=2))
            b_exp = bctx.enter_context(tc.tile_pool(name="b_exp", bufs=3))
            b_r = bctx.enter_context(tc.tile_pool(name="b_r", bufs=3))
            b_h3 = bctx.enter_context(tc.tile_pool(name="b_h3", bufs=2))
            spsb = bctx.enter_context(tc.tile_pool(name="spsb", bufs=1, space="PSUM"))
            odpb = bctx.enter_context(tc.tile_pool(name="odpb", bufs=2, space="PSUM"))
            svp = bctx.enter_context(tc.tile_pool(name="svp", bufs=1, space="PSUM"))
            swdp = bctx.enter_context(tc.tile_pool(name="swdp", bufs=1, space="PSUM"))

            def bh3_shift(d):
                l0 = d * 16
                bh3k = _mk(b_h3, [32, 2048], F16, "bh3k")
                bh3q = _mk(b_h3, [32, 2048], F16, "bh3q")
                src_k = s_kT_res[96:128, :].rearrange(
                    "p (g x) -> p g x", g=2)[:, :, l0 * 64: l0 * 64 + 1024]
                src_q = s_qT_res[96:128, :].rearrange(
                    "p (g x) -> p g x", g=2)[:, :, l0 * 64: l0 * 64 + 1024]
                nc.sync.dma_start(bh3k[:], src_k)
                nc.scalar.dma_start(bh3q[:], src_q)
                return bh3q, bh3k

            def swd_groups(d, s_att, tail=False):
                outst = [_mk(b_out, [128, CHW], F16, f"os{e}") for e in range(2)]
                groups = []

                def mk(e, j):
                    def run():
                        pool = swdp if (2 * e + j) % 2 == 0 or not tail else odpb
                        ps = _mk(pool, [128, 512], F32,
                                 "swdps" if pool is swdp else "odb")
                        for kt in range(2):
                            nc.tensor.matmul(
                                ps[:],
                                w_sb["s_wd"][kt][:, e * 128:(e + 1) * 128],
                                s_att[:, kt * 1024 + j * 512: kt * 1024 + (j + 1) * 512],
                                start=(kt == 0), stop=(kt == 1))
                        nc.vector.tensor_scalar_add(
                            outst[e][:, j * 512:(j + 1) * 512],
                            ps[:], s_bd_sb[:, e:e + 1])
                    return run

                def mk_dma(e):
                    def run():
                        nc.sync.dma_start(
                            outT.ap()[e * 128:(e + 1) * 128,
                                      d * CHW:(d + 1) * CHW], outst[e][:])
                    return run

                for j in range(2):
                    for e in range(2):
                        groups.append(mk(e, j))
                for e in range(2):
                    groups.append(mk_dma(e))
                return groups

            psb = _mk(spsb, [128, 2048], F32, "Sb")
            psvt = _mk(svp, [128, 512], F32, "psv")
            sv_all = {}
            s_atts = {}
            bh3s = {0: bh3_shift(0)}
            NST = 64

            def presv(g):
                w0 = g * 128
                psv = psvt[:, (g % 2) * 256:(g % 2) * 256 + 256]
                for kt in range(2):
                    nc.tensor.matmul(psv,
                                     vres[kt][:, w0: w0 + 128],
                                     w_sb["s_wv"][kt][:, :],
                                     start=(kt == 0), stop=(kt == 1))
                sv_all[g] = _mk(b_sv, [128, 256], F16, "sv")
                if g % 2 == 0:
                    nc.scalar.copy(sv_all[g][:], psv)
                else:
                    nc.vector.tensor_copy(sv_all[g][:], psv)

            def emit_front_b(g):
                d, u = divmod(g, 8)
                bh3q, bh3k = bh3s[d]
                w0 = g * 128
                par = g % 2

                def score(h):
                    hh, rr = h % 4, h // 4
                    if hh < 3:
                        lhsT = s_kT_res[32 * hh:32 * hh + 32,
                                        rr * T + w0: rr * T + w0 + 128]
                        rhs = s_qT_res[32 * hh:32 * hh + 32,
                                       rr * T + w0: rr * T + w0 + 128]
                    else:
                        lw = 2 * u * 64
                        lhsT = bh3k[0:32, rr * 1024 + lw: rr * 1024 + lw + 128]
                        rhs = bh3q[0:32, rr * 1024 + lw: rr * 1024 + lw + 128]
                    base = hh * 512 + par * 256 + rr * 128
                    nc.tensor.matmul(psb[:, base: base + 128], lhsT, rhs)

                for h in range(8):
                    score(h)
                ex = expP2[g % 5]
                for ql in range(2):
                    srcv = psb[64 * ql:64 * ql + 64, :].rearrange(
                        "p (hh x) -> p hh x", hh=4)[
                        :, :, par * 256:par * 256 + 256].rearrange(
                        "p hh (rr q) -> p hh rr q", q=128)[
                        :, :, :, ql * 64:ql * 64 + 64]
                    dstv = ex[64 * ql:64 * ql + 64, :].rearrange(
                        "p (rr hh q) -> p hh rr q", rr=2, q=128)[
                        :, :, :, ql * 64:ql * 64 + 64]
                    nc.scalar.activation(dstv, srcv, EXP)
                if g + 2 < NST:
                    presv(g + 2)

            def emit_back_b(g):
                d, u = divmod(g, 8)
                psod = _mk(odpb, [128, 512], F32, "odb")
                exp_u = expP2[g % 5]
                sv_u = sv_all.pop(g)
                for cc in range(4):
                    rhs = exp_u[:].rearrange("p (rr q) -> p rr q", rr=2)[
                        :, :, cc * 128:(cc + 1) * 128]
                    nc.tensor.matmul(psod[32 * cc:32 * cc + 32, 256:512],
                                     ones_sb[:, 0:32], rhs,
                                     tile_position=(0, 32 * cc))
                for h in range(H):
                    rr, cc = h // 4, h % 4
                    nc.tensor.matmul(
                        psod[32 * cc:32 * cc + 32, rr * 128:(rr + 1) * 128],
                        sv_u[:, h * 32:(h + 1) * 32],
                        exp_u[:, h * 128:(h + 1) * 128],
                        tile_position=(0, 32 * cc))
                rec = _mk(b_r, [128, 256], F32, "rec2")
                nc.vector.reciprocal(rec[:], psod[:, 256:512])
                dst = s_atts[d][:].rearrange("p (gg x) -> p gg x", gg=2)[
                    :, :, u * 128:(u + 1) * 128]
                po = psod[:, 0:256].rearrange("p (gg q) -> p gg q", gg=2)
                rc = rec[:].rearrange("p (gg q) -> p gg q", gg=2)
                nc.vector.tensor_mul(dst, po, rc)

            pending = []
            late = {}
            presv(0)
            presv(1)
            for g in range(NST + 2):
                d, u = divmod(g, 8)
                if g < NST and u == 0:
                    s_atts[d] = _mk(b_att, [128, 2048], F16, "sa")
                    if d > 0:
                        pending += swd_groups(d - 1, s_atts[d - 1])
                    if d + 1 < CH_A:
                        def prefetch(dd=d + 1):
                            bh3s[dd] = bh3_shift(dd)
                        pending.append(prefetch)
                    if d == CH_A - 1:
                        gl = swd_groups(d, s_atts[d], tail=True)
                        late = {62: [gl[0]], 63: [gl[1]],
                                65: [gl[2], gl[3], gl[4], gl[5]]}
                if g < NST:
                    emit_front_b(g)
                if g >= 2:
                    emit_back_b(g - 2)
                if pending:
                    pending.pop(0)()
                for gg in late.pop(g, []):
                    gg()
            while pending:
                pending.pop(0)()

    nc.compile()
    return nc


def pack_weights(t_wq, t_wk, t_wv, t_wd, t_bd, s_wq, s_wk, s_wv, s_wd, s_bd):
    s = 1.0 / np.sqrt(DK)
    t_wq_s = (np.asarray(t_wq, np.float64) * s)
    s_wq_s = (np.asarray(s_wq, np.float64) * s)
    sq_w = s_wq_s @ np.asarray(t_wd, np.float64)
    sq_bv = s_wq_s @ np.asarray(t_bd, np.float64)
    c = np.ascontiguousarray
    f16 = lambda a: c(np.asarray(a, np.float64).T.astype(np.float16))
    return {
        "t_wq": f16(t_wq_s), "t_wk": f16(t_wk), "t_wv": f16(t_wv),
        "sq_w": f16(sq_w),
        "s_wk": f16(s_wk), "s_wv": f16(s_wv), "s_wd": f16(s_wd),
        "sq_b": c(sq_bv.astype(np.float32)), "s_bd": c(np.asarray(s_bd, np.float32)),
    }


def pack_core_inputs(q_b, k_b, v_b, weights):
    qTh = np.ascontiguousarray(q_b.transpose(2, 1, 0).reshape(D, T).astype(np.float16))
    kTh = np.ascontiguousarray(k_b.transpose(2, 1, 0).reshape(D, T).astype(np.float16))
    vTh = np.ascontiguousarray(v_b.transpose(2, 0, 1).reshape(D, T).astype(np.float16))
    vTtmh = np.ascontiguousarray(v_b.transpose(2, 1, 0).reshape(D, T).astype(np.float16))
    return {"qT": qTh, "kT": kTh, "vT": vTh, "vTtm": vTtmh, **weights}


def unpack_core_output(outT_np):
    return np.ascontiguousarray(
        outT_np.astype(np.float32).reshape(D, L, NTR).transpose(1, 2, 0))


def ref_core(q_b, k_b, v_b, t_wq, t_wk, t_wv, t_wd, t_bd, s_wq, s_wk, s_wv, s_wd, s_bd):
    def lin(x, w, b=None):
        y = x @ w.T
        return y if b is None else y + b

    def sdpa(q, k, v):
        s = (q @ np.swapaxes(k, -1, -2)) / np.sqrt(q.shape[-1])
        s = s - s.max(-1, keepdims=True)
        p = np.exp(s)
        p = p / p.sum(-1, keepdims=True)
        return p @ v

    Lq, N, Dm = q_b.shape
    qt = np.swapaxes(q_b, 0, 1)
    kt = np.swapaxes(k_b, 0, 1)
    vt = np.swapaxes(v_b, 0, 1)
    qt = lin(qt, t_wq).reshape(N, Lq, H, DK).transpose(0, 2, 1, 3)
    kt = lin(kt, t_wk).reshape(N, Lq, H, DK).transpose(0, 2, 1, 3)
    vt = lin(vt, t_wv).reshape(N, Lq, H, DK).transpose(0, 2, 1, 3)
    x = sdpa(qt, kt, vt).transpose(0, 2, 1, 3).reshape(N, Lq, Dm)
    t_out = lin(np.swapaxes(x, 0, 1), t_wd, t_bd)

    qs = lin(t_out, s_wq).reshape(Lq, N, H, DK).transpose(0, 2, 1, 3)
    ks = lin(k_b, s_wk).reshape(Lq, N, H, DK).transpose(0, 2, 1, 3)
    vs = lin(v_b, s_wv).reshape(Lq, N, H, DK).transpose(0, 2, 1, 3)
    x = sdpa(qs, ks, vs).transpose(0, 2, 1, 3).reshape(Lq, N, Dm)
    return lin(x, s_wd, s_bd)


import os as _os

for _p in ("/opt/trn_rl_repo", _os.path.expanduser("~/.axon_site/_ro/trn_rl_repo")):
    if _os.path.isdir(_p) and _p not in sys.path:
        sys.path.insert(0, _p)

_NC_CACHE = {}


def _get_nc():
    if "nc" not in _NC_CACHE:
        _NC_CACHE["nc"] = build_nc(phases=_os.environ.get("KV3_PHASES", "AB"))
    return _NC_CACHE["nc"]


def _get_executor():
    if "exec" in _NC_CACHE:
        return _NC_CACHE["exec"]
    import jax
    from jax.sharding import Mesh, PartitionSpec, NamedSharding
    from jax.experimental.shard_map import shard_map
    from concourse.bass2jax import (_bass_exec_p, partition_id_tensor,
                                    install_neuronx_cc_hook)

    install_neuronx_cc_hook()
    nc = _get_nc()
    partition_name = nc.partition_id_tensor.name if nc.partition_id_tensor else None
    in_names, out_names, out_avals, zero_outs = [], [], [], []
    for alloc in nc.m.functions[0].allocations:
        if not isinstance(alloc, mybir.MemoryLocationSet):
            continue
        name = alloc.memorylocations[0].name
        if alloc.kind == "ExternalInput" and name != partition_name:
            in_names.append(name)
        elif alloc.kind == "ExternalOutput":
            out_names.append(name)
            shape = tuple(alloc.tensor_shape)
            dtype = mybir.dt.np(alloc.dtype)
            out_avals.append(jax.core.ShapedArray(shape, dtype))
            zero_outs.append(np.zeros(shape, dtype))
    all_names = list(in_names) + out_names
    if partition_name:
        all_names.append(partition_name)

    def _body(*args):
        operands = list(args)
        if partition_name is not None:
            operands.append(partition_id_tensor())
        return tuple(_bass_exec_p.bind(
            *operands, out_avals=tuple(out_avals), in_names=tuple(all_names),
            out_names=tuple(out_names), lowering_input_output_aliases=(),
            sim_require_finite=True, sim_require_nnan=True, nc=nc))

    devices = None
    for plat in ("axon", "neuron", None):
        try:
            devices = (jax.devices(plat) if plat else jax.devices())[:8]
            if len(devices) >= 8:
                break
        except RuntimeError:
            continue
    assert devices is not None and len(devices) >= 8, "need 8 neuron cores"
    mesh = Mesh(np.asarray(devices), ("core",))
    sh = NamedSharding(mesh, PartitionSpec("core"))
    nspec = len(in_names) + len(out_names)
    sharded = jax.jit(shard_map(_body, mesh=mesh,
                                in_specs=(PartitionSpec("core"),) * nspec,
                                out_specs=(PartitionSpec("core"),) * len(out_names),
                                check_rep=False), keep_unused=True)
    zeros_d = [jax.device_put(np.zeros((8 * z.shape[0], *z.shape[1:]), z.dtype), sh)
               for z in zero_outs]
    _NC_CACHE["exec"] = (sharded, in_names, out_names, sh, zeros_d, jax)
    return _NC_CACHE["exec"]


def kernel(query, key, value,
           t_wq, t_wk, t_wv, t_wd, t_bd,
           s_wq, s_wk, s_wv, s_wd, s_bd):
    query = np.asarray(query, dtype=np.float32)
    key = np.asarray(key, dtype=np.float32)
    value = np.asarray(value, dtype=np.float32)
    w = pack_weights(np.asarray(t_wq, np.float32), np.asarray(t_wk, np.float32),
                     np.asarray(t_wv, np.float32), np.asarray(t_wd, np.float32),
                     np.asarray(t_bd, np.float32), np.asarray(s_wq, np.float32),
                     np.asarray(s_wk, np.float32), np.asarray(s_wv, np.float32),
                     np.asarray(s_wd, np.float32), np.asarray(s_bd, np.float32))
    B = query.shape[0]
    assert B == 8, f"expected batch 8, got {B}"
    in_maps = [pack_core_inputs(query[b], key[b], value[b], w) for b in range(B)]
    sharded, in_names, out_names, sh, zeros_d, jax = _get_executor()
    args_d = []
    for nm in in_names:
        cat = np.concatenate([np.asarray(in_maps[c][nm]) for c in range(B)], axis=0)
        args_d.append(jax.device_put(cat, sh))
    outs = sharded(*args_d, *zeros_d)
    oi = out_names.index("outT")
    full = np.asarray(outs[oi]).reshape(8, D, T)
    return np.stack([unpack_core_output(full[b]) for b in range(B)])
